# revision 5
# baseline (speedup 1.0000x reference)
"""9x9 morphological dilation (sliding-window max, SAME padding) on Trainium2.

Input : label (16, 1024, 1024, 1) float32, values in [0, 1).
Output: same shape; out[b,i,j] = max over the 9x9 window centered at (i,j),
        clipped to the image (cv2-style border handling for dilate).

Strategy (per NeuronCore; batch is data-parallel over 8 cores, 2 images/core):
  - SBUF tile layout: 128 partitions x (16 rows x U cols).  Partition p holds
    img = p//64, row-block q = p%64 (image rows 16q..16q+15).  The free dim is
    r-major with U = 268 padded columns per chunk (4 col-chunks of 256 output
    cols, +-4 halo, zero pads at image edges; zero is a valid -inf substitute
    because all values are >= 0).
  - Vertical 9-max: log tree (shifts +1,+2,+4,+1 rows) as free-dim-shifted
    tensor_max ops; the 16-row block boundaries are fed by small SBUF->SBUF
    partition-shifted DMA "carry" tiles (DVE cannot read shifted partitions).
  - Horizontal 9-max: van Herk / Gil-Werman with two masked
    tensor_tensor_scan ops (segmented running max, segment length 9; the mask
    multiplies the running state by 0 at block starts) + one merge tensor_max.
  - The vertical result R9[r] covers rows R..R+8, i.e. output row R+4; the
    recentering happens in the store DMA offsets.  Output rows 0..3 (clipped
    top windows) are built from tree intermediates and stashed into the
    otherwise-unused rows (p%64==63, r>=12) so they ride the same horizontal
    pass and stores.
"""

import numpy as np

B, H, W = 16, 1024, 1024
NCORES = 8
IMGS = 2            # images per core
RB = 16             # rows per partition
Q = H // RB         # row blocks per image (64)
CW = 256            # output cols per chunk
NCH = W // CW       # 4 chunks
U = CW + 12         # padded cols per chunk: 4 left halo/pad + 256 + 8 right
F = RB * U          # free size of a full tile

_CACHE = {}


def _build(reps=1):
    import concourse.bacc as bacc
    import concourse.tile as tile
    import concourse.mybir as mybir

    f32 = mybir.dt.float32
    mx = mybir.AluOpType.max
    ml = mybir.AluOpType.mult

    nc = bacc.Bacc("TRN2", target_bir_lowering=False, debug=False, num_devices=1)
    x = nc.dram_tensor("x", [IMGS, H, W], f32, kind="ExternalInput").ap()
    y = nc.dram_tensor("y", [IMGS, H, W], f32, kind="ExternalOutput").ap()

    xv = [x[i].rearrange("(q r) c -> q r c", r=RB) for i in range(IMGS)]

    with tile.TileContext(nc) as tc:
        with (
            tc.tile_pool(name="px", bufs=2) as px,
            tc.tile_pool(name="pa", bufs=1) as pa,
            tc.tile_pool(name="pb", bufs=1) as pb,
            tc.tile_pool(name="pd", bufs=1) as pd,
            tc.tile_pool(name="ptop", bufs=1) as ptop,
            tc.tile_pool(name="pconst", bufs=1) as pconst,
            tc.tile_pool(name="pcarry", bufs=2) as pcarry,
        ):
            # --- persistent masks for the segmented horizontal scans ---
            # Mp: 0.0 where u % 268 == 4 + 9k (prefix-scan block starts)
            # Ms: 0.0 where u % 268 == 3 + 9k (suffix-scan block ends)
            MP = pconst.tile([128, F], f32, tag="mp")
            MS = pconst.tile([128, F], f32, tag="ms")
            mp3 = MP.rearrange("p (r u) -> p r u", u=U)
            ms3 = MS.rearrange("p (r u) -> p r u", u=U)
            nc.vector.memset(MP[:], 1.0)
            nc.vector.memset(MS[:], 1.0)
            # 30 block anchors per row: 4+9k <= 265 / 3+9k <= 264
            nc.vector.memset(mp3[:, :, 4:266:9], 0.0)
            nc.vector.memset(ms3[:, :, 3:265:9], 0.0)

            # --- persistent carry tiles (2 ping-pong sets) ---
            # rows 63 and 127 stay zero (image-bottom clamp)
            carr = []
            for s in range(2):
                xc1 = pconst.tile([128, 1 * U], f32, tag=f"xc1_{s}")
                t2c = pconst.tile([128, 2 * U], f32, tag=f"t2c_{s}")
                t4c = pconst.tile([128, 4 * U], f32, tag=f"t4c_{s}")
                t8c = pconst.tile([128, 1 * U], f32, tag=f"t8c_{s}")
                for t in (xc1, t2c, t4c, t8c):
                    nc.vector.memset(t[:], 0.0)
                carr.append((xc1, t2c, t4c, t8c))

            for ch in range(NCH * reps):
                ch = ch % NCH
                c0 = CW * ch          # first output col of this chunk
                # loaded cols: c = c0 - 4 + u for u in [ulo, ulo+ncols)
                clo = max(0, c0 - 4)
                chi = min(W, c0 + CW + 8)
                ncols = chi - clo
                ulo = clo - (c0 - 4)

                X = px.tile([128, F], f32, tag="x")
                x3 = X.rearrange("p (r u) -> p r u", u=U)
                # zero the (possible) unloaded pads, then load
                nc.vector.memset(x3[:, :, 0:4], 0.0)
                nc.vector.memset(x3[:, :, CW + 4:U], 0.0)
                for img in range(IMGS):
                    b = 64 * img
                    nc.sync.dma_start(
                        out=x3[b:b + 64, :, ulo:ulo + ncols],
                        in_=xv[img][:, :, clo:chi],
                    )

                xc1, t2c, t4c, t8c = carr[ch % 2]
                xc1_3 = xc1.rearrange("p (r u) -> p r u", u=U)
                t2c_3 = t2c.rearrange("p (r u) -> p r u", u=U)
                t4c_3 = t4c.rearrange("p (r u) -> p r u", u=U)
                t8c_3 = t8c.rearrange("p (r u) -> p r u", u=U)

                def carry_copy(dst3, src3, nrows):
                    # dst[p] = src[p+1, 0:nrows] for p in 0..62 and 64..126
                    nc.sync.dma_start(out=dst3[0:63, 0:nrows, :], in_=src3[1:64, 0:nrows, :])
                    nc.sync.dma_start(out=dst3[64:127, 0:nrows, :], in_=src3[65:128, 0:nrows, :])

                # --- vertical tree ---
                T2 = pa.tile([128, F], f32, tag="a")
                t2_3 = T2.rearrange("p (r u) -> p r u", u=U)
                carry_copy(xc1_3, x3, 1)
                nc.vector.tensor_max(t2_3[:, 0:15, :], x3[:, 0:15, :], x3[:, 1:16, :])
                nc.vector.tensor_max(t2_3[:, 15:16, :], x3[:, 15:16, :], xc1_3[:, 0:1, :])

                T4 = pb.tile([128, F], f32, tag="b")
                t4_3 = T4.rearrange("p (r u) -> p r u", u=U)
                carry_copy(t2c_3, t2_3, 2)
                nc.vector.tensor_max(t4_3[:, 0:14, :], t2_3[:, 0:14, :], t2_3[:, 2:16, :])
                nc.vector.tensor_max(t4_3[:, 14:16, :], t2_3[:, 14:16, :], t2c_3[:, 0:2, :])

                # top output rows 0..2 need X/T2/T4 of the first row-block
                TOP = ptop.tile([128, 4 * U], f32, tag="top")
                top3 = TOP.rearrange("p (r u) -> p r u", u=U)
                for img in range(IMGS):
                    b = 64 * img
                    nc.vector.tensor_max(top3[b:b + 1, 0:1, :], t4_3[b:b + 1, 0:1, :], x3[b:b + 1, 4:5, :])
                    nc.vector.tensor_max(top3[b:b + 1, 1:2, :], t4_3[b:b + 1, 0:1, :], t2_3[b:b + 1, 4:5, :])
                    nc.vector.tensor_max(top3[b:b + 1, 2:3, :], t4_3[b:b + 1, 0:1, :], t4_3[b:b + 1, 3:4, :])

                T8 = pa.tile([128, F], f32, tag="a")
                t8_3 = T8.rearrange("p (r u) -> p r u", u=U)
                carry_copy(t4c_3, t4_3, 4)
                nc.vector.tensor_max(t8_3[:, 0:12, :], t4_3[:, 0:12, :], t4_3[:, 4:16, :])
                nc.vector.tensor_max(t8_3[:, 12:16, :], t4_3[:, 12:16, :], t4c_3[:, 0:4, :])

                for img in range(IMGS):
                    b = 64 * img
                    nc.vector.tensor_copy(out=top3[b:b + 1, 3:4, :], in_=t8_3[b:b + 1, 0:1, :])

                R9 = pb.tile([128, F], f32, tag="b")
                r9_3 = R9.rearrange("p (r u) -> p r u", u=U)
                carry_copy(t8c_3, t8_3, 1)
                nc.vector.tensor_max(r9_3[:, 0:15, :], t8_3[:, 0:15, :], t8_3[:, 1:16, :])
                nc.vector.tensor_max(r9_3[:, 15:16, :], t8_3[:, 15:16, :], t8c_3[:, 0:1, :])

                # stash the 4 top output rows into the unused bottom-block rows
                for img in range(IMGS):
                    b = 64 * img
                    nc.sync.dma_start(out=r9_3[b + 63:b + 64, 12:16, :], in_=top3[b:b + 1, 0:4, :])

                # --- horizontal van Herk ---
                PH = pa.tile([128, F], f32, tag="a")
                SH = pd.tile([128, F], f32, tag="d")
                nc.vector.tensor_tensor_scan(PH[:], MP[:], R9[:], 0.0, op0=ml, op1=mx)
                nc.vector.tensor_tensor_scan(
                    SH[:, ::-1], MS[:, ::-1], R9[:, ::-1], 0.0, op0=ml, op1=mx
                )

                OUT = pb.tile([128, F], f32, tag="b")
                o3 = OUT.rearrange("p (r u) -> p r u", u=U)
                ph3 = PH.rearrange("p (r u) -> p r u", u=U)
                sh3 = SH.rearrange("p (r u) -> p r u", u=U)
                nc.vector.tensor_max(o3[:, :, 4:4 + CW], sh3[:, :, 0:CW], ph3[:, :, 8:8 + CW])

                # --- stores (recenter: R9 row R -> output row R+4) ---
                for img in range(IMGS):
                    b = 64 * img
                    ymain = y[img][4:4 + 63 * RB, c0:c0 + CW].rearrange(
                        "(q r) c -> q r c", r=RB
                    )
                    nc.sync.dma_start(out=ymain, in_=o3[b:b + 63, :, 4:4 + CW])
                    ytail = y[img][4 + 63 * RB:H, c0:c0 + CW]
                    nc.sync.dma_start(out=ytail, in_=o3[b + 63:b + 64, 0:12, 4:4 + CW])
                    ytop = y[img][0:4, c0:c0 + CW]
                    nc.sync.dma_start(out=ytop, in_=o3[b + 63:b + 64, 12:16, 4:4 + CW])

    nc.compile()
    return nc


def kernel(label):
    lab = np.ascontiguousarray(
        np.asarray(label, dtype=np.float32).reshape(B, H, W)
    )
    if "nc" not in _CACHE:
        _CACHE["nc"] = _build()
    nc = _CACHE["nc"]

    from concourse.bass_utils import run_bass_kernel_spmd

    in_maps = [{"x": lab[IMGS * c:IMGS * (c + 1)]} for c in range(NCORES)]
    res = run_bass_kernel_spmd(nc, in_maps, core_ids=list(range(NCORES)))
    out = np.concatenate([res.results[c]["y"] for c in range(NCORES)], axis=0)
    return out.reshape(B, H, W, 1)
